# revision 1
# baseline (speedup 1.0000x reference)
"""ChannelWiseProjection Trainium2 kernel.

out[b,c,h,w] = sum_d x[b,h,w,d] * W[c,d] + bias[c]

Strategy: data-parallel over M = b*h*w (65536 rows), 8192 rows per core.
Host pre-transposes each core's x slab to [D=512, M=8192] (K-major) so the
device sees the contraction dim on SBUF partitions with no on-chip
transpose.  Per core: out_slab[C=128, M=8192] = W^T-blocked stationary
matmuls (fp32r, 4 K-blocks accumulated in PSUM) + bias fused into the
PSUM->SBUF copy.  Output slabs are channel-major so they DMA straight out
and reassemble into [b, c, h, w] on host.
"""

import numpy as np

from concourse import bacc, mybir, tile
from concourse.bass_utils import run_bass_kernel_spmd

N_CORES = 8
B, H, Wdim, D = 4, 128, 128, 512
C = 128
M_TOT = B * H * Wdim          # 65536
M_CORE = M_TOT // N_CORES     # 8192
KB = D // 128                 # 4 contraction blocks
M_SUB = 512                   # matmul moving size (one PSUM bank, fp32)
# Chunk schedule along M.  Small first chunk starts the compute/store
# pipeline early; small final chunks minimize the residual work that
# serializes after the last load byte lands (DMA is the binding resource,
# so the final load always ends at ~total_bytes/fabric_bw regardless).
CHUNKS = [256, 512] + [1024] * 7 + [256]
assert sum(CHUNKS) == M_CORE

_NC = None


def _build():
    global _NC
    if _NC is not None:
        return _NC
    # Bacc (not raw Bass): its finalize() runs the pass pipeline that
    # splits multi-waits into EventSemaphores (TRN2 allows only one sync
    # wait per instruction) — Tile output does not compile without it.
    nc = bacc.Bacc(None)
    xt = nc.declare_dram_parameter(
        "xt", [KB, 128, M_CORE], mybir.dt.float32r, isOutput=False
    )
    wt = nc.declare_dram_parameter(
        "wt", [128, KB, C], mybir.dt.float32r, isOutput=False
    )
    bias = nc.declare_dram_parameter("bias", [C, 1], mybir.dt.float32, isOutput=False)
    out = nc.declare_dram_parameter("out", [C, M_CORE], mybir.dt.float32, isOutput=True)

    with tile.TileContext(nc) as tc:
        with (
            tc.tile_pool(name="const", bufs=1) as cpool,
            tc.tile_pool(name="x", bufs=8) as xpool,
            tc.tile_pool(name="o", bufs=10) as opool,
            tc.tile_pool(name="ps", bufs=8, space="PSUM") as pspool,
        ):
            # w/bias ride the ACT HWDGE ring, which is idle until the first
            # store (~19us) — they land ~4us earlier than via SWDGE, and the
            # first matmul is gated on w's arrival.
            w_sb = cpool.tile([128, KB, C], mybir.dt.float32r)
            nc.scalar.dma_start(w_sb[:], wt[:])
            b_sb = cpool.tile([C, 1], mybir.dt.float32)
            nc.scalar.dma_start(b_sb[:], bias[:])

            xt_r = xt[:].rearrange("kb p m -> p kb m")
            off = 0
            for size in CHUNKS:
                x_sb = xpool.tile([128, KB, size], mybir.dt.float32r)
                nc.sync.dma_start(x_sb[:], xt_r[:, :, off : off + size])
                o_sb = opool.tile([C, size], mybir.dt.float32)
                for ms0 in range(0, size, M_SUB):
                    sub = min(M_SUB, size - ms0)
                    ps = pspool.tile([C, sub], mybir.dt.float32)
                    for kb in range(KB):
                        nc.tensor.matmul(
                            ps[:],
                            w_sb[:, kb, :],
                            x_sb[:, kb, ms0 : ms0 + sub],
                            start=(kb == 0),
                            stop=(kb == KB - 1),
                        )
                    nc.vector.tensor_scalar_add(
                        o_sb[:, ms0 : ms0 + sub], ps[:], b_sb[:]
                    )
                # Stores ride the ACT HWDGE ring so they never queue behind
                # the loads on the SP ring.
                nc.scalar.dma_start(out[:, off : off + size], o_sb[:])
                off += size
    nc.finalize()  # Bacc.finalize runs the wait-splitting compile pipeline
    _NC = nc
    return nc


LAST_RESULT = None


def kernel(x, W, b):
    global LAST_RESULT
    nc = _build()

    x = np.ascontiguousarray(np.asarray(x), dtype=np.float32)
    W = np.asarray(W, dtype=np.float32)
    b = np.asarray(b, dtype=np.float32)

    # Per-core K-major slabs: [8, D, M_CORE] -> [8, KB, 128, M_CORE]
    xt = np.ascontiguousarray(
        x.reshape(N_CORES, M_CORE, D).transpose(0, 2, 1)
    ).reshape(N_CORES, KB, 128, M_CORE)
    # Stationary weights, blocked: wt[kp, kb, c] = W[c, kb*128 + kp]
    wt = np.ascontiguousarray(W.T.reshape(KB, 128, C).transpose(1, 0, 2))
    b2 = np.ascontiguousarray(b.reshape(C, 1))

    import os

    in_maps = [{"xt": xt[i], "wt": wt, "bias": b2} for i in range(N_CORES)]
    res = None
    for attempt in range(4):
        try:
            if attempt == 0:
                res = run_bass_kernel_spmd(nc, in_maps, list(range(N_CORES)))
            else:
                # Retry without NTFF tracing: the profile hook's client
                # handle is stale after a backend reset and would raise
                # before the exec even runs.
                os.environ["BASS_NEVER_TRACE"] = "1"
                try:
                    res = run_bass_kernel_spmd(nc, in_maps, list(range(N_CORES)))
                finally:
                    os.environ.pop("BASS_NEVER_TRACE", None)
            break
        except Exception:
            # Transient NRT_EXEC_UNIT_UNRECOVERABLE wedges (stale device
            # state left by a previous process) clear after a backend reset.
            if attempt == 3:
                raise
            try:
                import jax

                jax.clear_caches()
                jax.extend.backend.clear_backends()
                jax.devices()
            except Exception:
                pass
    LAST_RESULT = res

    out = np.empty((B, C, H, Wdim), dtype=np.float32)
    for i in range(N_CORES):
        slab = res.results[i]["out"]  # [C, M_CORE] over m = (h, w) for batch i//2
        bi, half = divmod(i, 2)
        out[bi, :, half * 64 : (half + 1) * 64, :] = slab.reshape(C, 64, Wdim)
    return out



# revision 2
# speedup vs baseline: 1.5671x; 1.5671x over previous
"""ChannelWiseProjection Trainium2 kernel.

out[b,c,h,w] = sum_d x[b,h,w,d] * W[c,d] + bias[c]

Strategy: data-parallel over M = b*h*w (65536 rows), 8192 rows per core.
The tolerance (2e-2) leaves ~8x headroom for bf16, so the host casts x to
bf16 (halving load traffic vs fp32) and the device stores bf16 outputs
(halving store traffic); the host upcasts to fp32.  Per core the DMA
floor is 8.39MB load + 2.1MB store ~= 29us at 360 GB/s.

Layout: host packs each chunk of M into its own contiguous DRAM tensor
[128, KB, mc] (k = p*KB + kb on partitions) so each chunk load is one
DIRECT2D with 128 contiguous per-partition descriptors.  All SBUF tiles
are resident (no pool rotation -> fewer semaphores, shorter sequencer
drain tail).  Chunks shrink toward the end so the serialized
work after the last load byte (matmul+bias+store of the final chunk) is
small.
"""

import numpy as np
import ml_dtypes

from concourse import bacc, mybir, tile
from concourse.bass_utils import run_bass_kernel_spmd

N_CORES = 8
B, H, Wdim, D = 4, 128, 128, 512
C = 128
M_TOT = B * H * Wdim          # 65536
M_CORE = M_TOT // N_CORES     # 8192
KB = D // 128                 # 4 contraction blocks
M_SUB = 512                   # PSUM bank width in fp32
CHUNKS = [2048, 2048, 2048, 1024, 512, 256, 256]
assert sum(CHUNKS) == M_CORE

BF16 = mybir.dt.bfloat16

_NC = None


def _build():
    global _NC
    if _NC is not None:
        return _NC
    # Bacc (not raw Bass): its finalize() runs the pass pipeline that
    # splits multi-waits into EventSemaphores (TRN2 allows only one sync
    # wait per instruction) — Tile output does not compile without it.
    nc = bacc.Bacc(None)
    xts = [
        nc.declare_dram_parameter(f"x{i}", [128, KB, mc], BF16, isOutput=False)
        for i, mc in enumerate(CHUNKS)
    ]
    wt = nc.declare_dram_parameter("wt", [128, KB, C], BF16, isOutput=False)
    bias = nc.declare_dram_parameter("bias", [C, 1], mybir.dt.float32, isOutput=False)
    outs = [
        nc.declare_dram_parameter(f"o{i}", [C, mc], BF16, isOutput=True)
        for i, mc in enumerate(CHUNKS)
    ]

    with tile.TileContext(nc) as tc:
        with (
            tc.tile_pool(name="sb", bufs=1) as pool,
            tc.tile_pool(name="ps", bufs=8, space="PSUM") as pspool,
        ):
            # w/bias ride the ACT HWDGE ring (idle until the first store)
            # so they land ahead of the x stream; the first matmul is
            # gated on w's arrival.
            w_sb = pool.tile([128, KB, C], BF16, tag="w")
            nc.scalar.dma_start(w_sb[:], wt[:])
            b_sb = pool.tile([C, 1], mybir.dt.float32, tag="b")
            nc.scalar.dma_start(b_sb[:], bias[:])

            for i, mc in enumerate(CHUNKS):
                x_sb = pool.tile([128, KB, mc], BF16, tag=f"x{i}")
                nc.sync.dma_start(x_sb[:], xts[i][:])
                o_sb = pool.tile([C, mc], BF16, tag=f"o{i}")
                for ms0 in range(0, mc, M_SUB):
                    sub = min(M_SUB, mc - ms0)
                    ps = pspool.tile([C, sub], mybir.dt.float32)
                    for kb in range(KB):
                        nc.tensor.matmul(
                            ps[:],
                            w_sb[:, kb, :],
                            x_sb[:, kb, ms0 : ms0 + sub],
                            start=(kb == 0),
                            stop=(kb == KB - 1),
                        )
                    nc.vector.tensor_scalar_add(
                        o_sb[:, ms0 : ms0 + sub], ps[:], b_sb[:]
                    )
                # Stores ride the ACT HWDGE ring so they never queue behind
                # the loads on the SP ring.
                nc.scalar.dma_start(outs[i][:], o_sb[:])
    nc.finalize()  # Bacc.finalize runs the wait-splitting compile pipeline
    _NC = nc
    return nc


LAST_RESULT = None


def kernel(x, W, b):
    global LAST_RESULT
    nc = _build()

    x = np.asarray(x, dtype=np.float32)
    W = np.asarray(W, dtype=np.float32)
    b = np.asarray(b, dtype=np.float32)

    # k = p*KB + kb on SBUF partitions: x[m, k] -> [128, KB, mc] per chunk.
    xbf = x.reshape(N_CORES, M_CORE, D).astype(ml_dtypes.bfloat16)
    # Stationary weights blocked the same way: wt[p, kb, c] = W[c, p*KB+kb]
    wt = np.ascontiguousarray(
        W.reshape(C, 128, KB).transpose(1, 2, 0).astype(ml_dtypes.bfloat16)
    )
    b2 = np.ascontiguousarray(b.reshape(C, 1))

    offs = np.cumsum([0] + CHUNKS)
    in_maps = []
    for core in range(N_CORES):
        m = {"wt": wt, "bias": b2}
        slab = xbf[core]  # [M_CORE, D]
        for i, mc in enumerate(CHUNKS):
            blk = slab[offs[i] : offs[i] + mc]          # [mc, D]
            m[f"x{i}"] = np.ascontiguousarray(
                blk.reshape(mc, 128, KB).transpose(1, 2, 0)
            )
        in_maps.append(m)

    import os

    res = None
    for attempt in range(4):
        try:
            if attempt == 0:
                res = run_bass_kernel_spmd(nc, in_maps, list(range(N_CORES)))
            else:
                # Retry without NTFF tracing: the profile hook's client
                # handle is stale after a backend reset and would raise
                # before the exec even runs.
                os.environ["BASS_NEVER_TRACE"] = "1"
                try:
                    res = run_bass_kernel_spmd(nc, in_maps, list(range(N_CORES)))
                finally:
                    os.environ.pop("BASS_NEVER_TRACE", None)
            break
        except Exception:
            # Transient NRT_EXEC_UNIT_UNRECOVERABLE wedges (stale device
            # state left by a previous process) clear after a backend reset.
            if attempt == 3:
                raise
            try:
                import jax

                jax.clear_caches()
                jax.extend.backend.clear_backends()
                jax.devices()
            except Exception:
                pass
    LAST_RESULT = res

    out = np.empty((B, C, H, Wdim), dtype=np.float32)
    for i in range(N_CORES):
        slab = np.concatenate(
            [np.asarray(res.results[i][f"o{j}"]) for j in range(len(CHUNKS))],
            axis=1,
        ).astype(np.float32)  # [C, M_CORE] over m = (h, w) for batch i//2
        bi, half = divmod(i, 2)
        out[bi, :, half * 64 : (half + 1) * 64, :] = slab.reshape(C, 64, Wdim)
    return out
